# revision 14
# baseline (speedup 1.0000x reference)
"""GCN layer (gather + segment-sum + degree norm) on 8 trn2 NeuronCores.

Sharding: destination nodes across cores (12500/core). Host computes
degree rsqrt factors from the index arrays (bincount) and packs edges;
devices do all float work on the embedding payload.

Launch 1 (per core): h2 = h * odr, f32 -> bf16, over the core's 12544-row
slice in 14 blocks of [128, 7*128]. Host concatenates slices into the
[100000, 128] bf16 gather table.

Launch 2 (per core): per super-window (7 windows of 128 dst lanes):
one indirect-DMA gather (or 4 chunked gpsimd.dma_gather calls) of 256B
bf16 rows into SBUF slabs; per window: one-hot [P, WN, Kw] via is_equal
in a 2x-DVE-friendly layout; Kw matmuls accumulate onehot^T @ rows into
PSUM [128, 128]; activation-engine copy with scale=idr writes the
scaled result; one DMA per super writes [128, 7*128] f32 out (p-major,
host transposes).
"""

import numpy as np
import ml_dtypes

import concourse.bass as bass
import concourse.bacc as bacc
import concourse.mybir as mybir
import concourse.tile as tile
from concourse.bass_utils import run_bass_kernel_spmd

f32 = mybir.dt.float32
bf16 = mybir.dt.bfloat16
i16 = mybir.dt.int16
i32 = mybir.dt.int32
BF = ml_dtypes.bfloat16

P = 128
D = 128

# gather mode: 'indirect' = one indirect_dma_start per super (int32 offsets,
# oob pads skipped); 'ucode' = gpsimd.dma_gather per (super, chunk) with
# int16 idxs into <=32768-row chunks.
GATHER_MODE = "ucode"
GATHER_MAXC = 8  # max 128-row columns per dma_gather call (1024 idx ucode cap)
GATHER_NQ = 4   # SWDGE queues for gather round-robin


class Cfg:
    def __init__(self, n_src, n_dst, n_cores, sw, chunk):
        self.N_SRC = n_src
        self.N_DST = n_dst
        self.N = n_src + n_dst
        self.C = n_cores
        self.NPC = self.N // n_cores          # dst nodes per core
        self.WN = 128                          # window = PSUM partition dim
        self.NW = -(-self.NPC // self.WN)      # windows per core
        self.SW = sw                           # windows per super
        assert self.NW % sw == 0
        self.NSUP = self.NW // sw
        self.NPC_PAD = self.NW * self.WN
        self.CHUNK = chunk
        nb = []
        b = 0
        while b < self.N:
            nb.append(b)
            b += chunk
        nb.append(self.N)
        self.CHUNKB = nb                       # chunk row boundaries
        self.NCH = len(nb) - 1


CFG = Cfg(60000, 40000, 8, 7, 32768)


# ---------------------------------------------------------------- host packing
def _pack2(cfg, src_idx, dst_idx, mode):
    """Pack edges for launch 2. Returns dict of per-core arrays + layout."""
    C, NPC, WN, NW, SW, NSUP = cfg.C, cfg.NPC, cfg.WN, cfg.NW, cfg.SW, cfg.NSUP
    NCH = cfg.NCH if mode == "ucode" else 1
    core = dst_idx // NPC
    dloc = dst_idx - core * NPC
    w = dloc // WN
    lane = dloc - w * WN
    if mode == "ucode":
        ch = np.minimum(src_idx // cfg.CHUNK, cfg.NCH - 1)
    else:
        ch = np.zeros_like(src_idx)

    # counts per (core, window, chunk); K uniform across cores
    counts = np.zeros((C, NW, NCH), dtype=np.int64)
    np.add.at(counts, (core, w, ch), 1)
    K = -(-counts.max(axis=0) // P)            # [NW, NCH]

    choff = np.zeros((NW, NCH), dtype=np.int64)  # col offset of (w,ch) within window
    np.cumsum(K[:, :-1], axis=1, out=choff[:, 1:])
    Kw = K.sum(axis=1)                            # [NW]
    wcol0 = np.concatenate([[0], np.cumsum(Kw)])  # window col base (w-major layout)
    wkcols = int(wcol0[-1])
    KMAX = int(Kw.max())

    # per-super per-chunk call sizes (ucode); msgs col layout
    # indirect: msgs cols == w-major cols within super.
    # ucode: msgs cols grouped (chunk-major within super, window-major within chunk)
    Kcall = np.zeros((NSUP, NCH), dtype=np.int64)
    for s in range(NSUP):
        Kcall[s] = K[s * SW:(s + 1) * SW].sum(axis=0)
    supcols = Kcall.sum(axis=1)                   # [NSUP]
    scol0 = wcol0[np.arange(NSUP) * SW]           # w-major col base of super

    # sort edges by (core, w, ch) stable; compute within-group index
    order = np.lexsort((ch, w, core))
    s_src = src_idx[order]
    s_lane = lane[order].astype(np.float32)
    s_core = core[order]
    s_w = w[order]
    s_ch = ch[order]

    gid = (s_core * NW + s_w) * NCH + s_ch        # group id, sorted ascending
    grp_counts = counts.reshape(-1)               # [(c,w,ch)] in same order
    grp_starts = np.concatenate([[0], np.cumsum(grp_counts)])
    within = np.arange(len(s_src)) - grp_starts[gid]

    # w-major placement (for dstl, and for msgs/offsets in indirect mode)
    col_w = wcol0[s_w] + choff[s_w, s_ch] + within // P
    part = within % P

    dstl = np.full((C, P, wkcols), -1.0, dtype=np.float32)
    dstl[s_core, part, col_w] = s_lane
    dstl = dstl.astype(BF)

    out = {
        "K": K, "Kw": Kw, "wcol0": wcol0, "choff": choff, "wkcols": wkcols,
        "KMAX": KMAX, "Kcall": Kcall, "supcols": supcols, "scol0": scol0,
        "dstl": dstl,
    }

    if mode == "indirect":
        offs = np.full((C, P, wkcols), np.int32(2**30), dtype=np.int32)
        offs[s_core, part, col_w] = s_src.astype(np.int32)
        out["offs"] = offs
        return out

    # ucode: idx slabs per (super, chunk) call, chunk-major msgs layout.
    # col within call = preK(s,w,ch) + within//P where preK = sum over w'<w
    # in super of K[w',ch].
    preK = np.zeros((NW, NCH), dtype=np.int64)
    for s in range(NSUP):
        blk = K[s * SW:(s + 1) * SW]
        preK[s * SW:(s + 1) * SW] = np.cumsum(blk, axis=0) - blk
    # msgs col base of (s, ch) within super tile
    callbase = np.zeros((NSUP, NCH), dtype=np.int64)
    np.cumsum(Kcall[:, :-1], axis=1, out=callbase[:, 1:])

    icol0 = np.zeros((NSUP, NCH), dtype=np.int64)  # idx col base (units of 8)
    iacc = 0
    for s in range(NSUP):
        for c2 in range(NCH):
            icol0[s, c2] = iacc
            iacc += Kcall[s, c2] * 8
    icols = iacc

    s_sup = s_w // SW
    j = (preK[s_w, s_ch] * P + within)            # slot within call
    idx_flat = np.zeros((C, NSUP, NCH), dtype=object)
    # build per-call idx lists vectorized: global slot position
    slot_base = np.zeros((C, NSUP, NCH), dtype=np.int64)
    for c3 in range(C):
        for s in range(NSUP):
            for c2 in range(NCH):
                slot_base[c3, s, c2] = (c3 * icols + icol0[s, c2]) * 2  # placeholder
    # simpler: fill a [C, icols*16] flat int16 array? idx slot j of call ->
    # wrapped [j%16, icol0 + j//16] replicated to 128 partitions.
    idxs = np.zeros((C, P, icols), dtype=np.int16)
    vals = (s_src - np.array(cfg.CHUNKB)[s_ch]).astype(np.int16)
    r16 = (j % 16).astype(np.int64)
    c16 = (j // 16).astype(np.int64)
    colpos = icol0[s_sup, s_ch] + c16
    for rep in range(8):
        idxs[s_core, rep * 16 + r16, colpos] = vals
    # replicate: above writes only [r16] rows per rep via fancy indexing
    out["idxs"] = idxs
    out["icol0"] = icol0
    out["icols"] = icols
    out["preK"] = preK
    out["callbase"] = callbase
    return out


def _degree_arrays(cfg, src_idx, dst_idx):
    od = np.bincount(src_idx, minlength=cfg.N).astype(np.float32)
    idg = np.bincount(dst_idx, minlength=cfg.N).astype(np.float32)
    odr = 1.0 / np.sqrt(np.maximum(od, 1.0))
    idr = 1.0 / np.sqrt(np.maximum(idg, 1.0))
    return odr, idr


def _per_core_pwd(cfg, vec, c):
    """[N] vector -> [P, NW] (p-minor) slice for core c, padded."""
    s = np.ones(cfg.NPC_PAD, dtype=np.float32)
    s[:cfg.NPC] = vec[c * cfg.NPC:(c + 1) * cfg.NPC]
    return np.ascontiguousarray(s.reshape(cfg.NW, P).T)


# ---------------------------------------------------------------- bass builders
def _build_phase1(cfg, repeat=1):
    nc = bacc.Bacc("TRN2", target_bir_lowering=False)
    NW, SW, NSUP = cfg.NW, cfg.SW, cfg.NSUP
    h_d = nc.dram_tensor("h_t", [P, NW, D], f32, kind="ExternalInput")
    odr_d = nc.dram_tensor("odr", [P, NW], f32, kind="ExternalInput")
    h2_d = nc.dram_tensor("h2s", [P, NW, D], bf16, kind="ExternalOutput")

    with tile.TileContext(nc) as tc:
        with (
            tc.tile_pool(name="cst", bufs=1) as cst,
            tc.tile_pool(name="wk", bufs=3) as wk,
        ):
            odr = cst.tile([P, NW], f32)
            nc.sync.dma_start(odr[:], odr_d[:])

            def body(_=None):
                for b in range(NSUP):
                    ht = wk.tile([P, SW, D], f32, tag="ht")
                    nc.sync.dma_start(ht[:], h_d[:, b * SW:(b + 1) * SW, :])
                    h2t = wk.tile([P, SW, D], bf16, tag="h2t")
                    nc.vector.tensor_tensor(
                        out=h2t[:],
                        in0=ht[:],
                        in1=odr[:, b * SW:(b + 1) * SW, None].to_broadcast(
                            [P, SW, D]
                        ),
                        op=mybir.AluOpType.mult,
                    )
                    nc.sync.dma_start(h2_d[:, b * SW:(b + 1) * SW, :], h2t[:])

            if repeat > 1:
                with tc.For_i(0, repeat, 1):
                    body()
            else:
                body()
    nc.compile()
    return nc


def _build_phase2(cfg, pk, mode, repeat=1, parts="all"):
    nc = bacc.Bacc("TRN2", target_bir_lowering=False, num_swdge_queues=4)
    NW, SW, NSUP, WN = cfg.NW, cfg.SW, cfg.NSUP, cfg.WN
    K, Kw, wcol0, choff = pk["K"], pk["Kw"], pk["wcol0"], pk["choff"]
    wkcols, KMAX, supcols, scol0 = pk["wkcols"], pk["KMAX"], pk["supcols"], pk["scol0"]

    h2_d = nc.dram_tensor("h2", [cfg.N, D], bf16, kind="ExternalInput")
    dstl_d = nc.dram_tensor("dstl", [P, wkcols], bf16, kind="ExternalInput")
    iota_d = nc.dram_tensor("iota3", [P, WN, KMAX], bf16, kind="ExternalInput")
    idr_d = nc.dram_tensor("idr", [P, NW], f32, kind="ExternalInput")
    if mode == "indirect":
        offs_d = nc.dram_tensor("offs", [P, wkcols], i32, kind="ExternalInput")
    else:
        idxs_d = nc.dram_tensor("idxs", [P, pk["icols"]], i16, kind="ExternalInput")
    out_d = nc.dram_tensor("out_t", [P, NW * D], f32, kind="ExternalOutput")

    MAXSUP = int(max(supcols))

    with tile.TileContext(nc) as tc:
        with (
            tc.tile_pool(name="cst", bufs=1) as cst,
            tc.tile_pool(name="msgs", bufs=3) as mp,
            tc.tile_pool(name="ohp", bufs=3) as ohp,
            tc.tile_pool(name="finp", bufs=2) as fp,
            tc.tile_pool(name="psum", bufs=4, space="PSUM") as ps,
        ):
            dstl = cst.tile([P, wkcols], bf16)
            nc.sync.dma_start(dstl[:], dstl_d[:])
            iota3 = cst.tile([P, WN, KMAX], bf16)
            nc.sync.dma_start(iota3[:], iota_d[:])
            idr = cst.tile([P, NW], f32)
            nc.sync.dma_start(idr[:], idr_d[:])
            if mode == "indirect":
                offs = cst.tile([P, wkcols], i32)
                nc.sync.dma_start(offs[:], offs_d[:])
            else:
                idxs = cst.tile([P, pk["icols"]], i16)
                nc.sync.dma_start(idxs[:], idxs_d[:])

            qctr = [0]

            def body(_=None):
                for s in range(NSUP):
                    sc = int(supcols[s])
                    msgs = mp.tile([P, MAXSUP, D], bf16, tag="m")
                    if mode == "indirect":
                        nc.gpsimd.indirect_dma_start(
                            out=msgs[:, 0:sc, :],
                            out_offset=None,
                            in_=h2_d[:, :],
                            in_offset=bass.IndirectOffsetOnAxis(
                                ap=offs[:, int(scol0[s]):int(scol0[s]) + sc],
                                axis=0,
                            ),
                            bounds_check=cfg.N - 1,
                            oob_is_err=False,
                        )
                    else:
                        for c2 in range(cfg.NCH):
                            Kc = int(pk["Kcall"][s, c2])
                            if Kc == 0:
                                continue
                            cb = int(pk["callbase"][s, c2])
                            ic = int(pk["icol0"][s, c2])
                            # DMA desc ring caps one call's descriptors
                            # (16 idx each) -> split into <=MAXC column
                            # chunks.
                            MAXC = GATHER_MAXC
                            for k0 in range(0, Kc, MAXC):
                                kn = min(MAXC, Kc - k0)
                                nc.gpsimd.dma_gather(
                                    out_ap=msgs[:, cb + k0:cb + k0 + kn, :],
                                    in_ap=h2_d[
                                        cfg.CHUNKB[c2]:cfg.CHUNKB[c2 + 1], :
                                    ],
                                    idxs_ap=idxs[
                                        :, ic + k0 * 8:ic + (k0 + kn) * 8
                                    ],
                                    num_idxs=kn * P,
                                    num_idxs_reg=kn * P,
                                    elem_size=D,
                                    queue_num=qctr[0] % GATHER_NQ,
                                )
                                qctr[0] += 1
                    if parts == "gather":
                        continue
                    fin = fp.tile([P, SW * D], f32, tag="fin")
                    for wi in range(SW):
                        w = s * SW + wi
                        Kw_w = int(Kw[w])
                        wc0 = int(wcol0[w])
                        if Kw_w == 0:
                            continue
                        oh = ohp.tile([P, WN, Kw_w], bf16, tag="oh")
                        nc.vector.tensor_tensor(
                            out=oh[:],
                            in0=dstl[:, None, wc0:wc0 + Kw_w].to_broadcast(
                                [P, WN, Kw_w]
                            ),
                            in1=iota3[:, :, 0:Kw_w],
                            op=mybir.AluOpType.is_equal,
                        )
                        if parts == "onehot":
                            continue
                        acc = ps.tile([WN, D], f32, space="PSUM")
                        kk = 0
                        for c2 in range(cfg.NCH if mode == "ucode" else 1):
                            if mode == "ucode":
                                Kwc = int(K[w, c2])
                                mbase = int(
                                    pk["callbase"][s, c2] + pk["preK"][w, c2]
                                )
                            else:
                                Kwc = Kw_w
                                mbase = wc0 - int(scol0[s])
                            for k in range(Kwc):
                                nc.tensor.matmul(
                                    acc[:],
                                    lhsT=oh[:, :, kk + k if mode == "ucode" else k],
                                    rhs=msgs[:, mbase + k, :],
                                    start=(kk + k == 0),
                                    stop=(kk + k == Kw_w - 1),
                                )
                            kk += Kwc
                        nc.scalar.activation(
                            out=fin[:, wi * D:(wi + 1) * D],
                            in_=acc[:],
                            func=mybir.ActivationFunctionType.Copy,
                            scale=idr[:, w:w + 1],
                        )
                    if parts == "onehot":
                        continue
                    nc.sync.dma_start(
                        out_d[:, s * SW * D:(s + 1) * SW * D], fin[:]
                    )

            if repeat > 1:
                with tc.For_i(0, repeat, 1):
                    body()
            else:
                body()
    nc.compile()
    return nc


# ---------------------------------------------------------------- driver
def _iota3_np(cfg, KMAX):
    a = np.arange(cfg.WN, dtype=np.float32)[None, :, None]
    return np.ascontiguousarray(
        np.broadcast_to(a, (P, cfg.WN, KMAX)).astype(BF)
    )


def _prepare(cfg, src_embedding, dst_embedding, src_idx, dst_idx, mode):
    src_idx = np.asarray(src_idx).astype(np.int64)
    dst_idx = np.asarray(dst_idx).astype(np.int64)
    h = np.concatenate(
        [np.asarray(src_embedding, np.float32), np.asarray(dst_embedding, np.float32)],
        axis=0,
    )
    odr, idr = _degree_arrays(cfg, src_idx, dst_idx)

    in1 = []
    for c in range(cfg.C):
        hs = np.zeros((cfg.NPC_PAD, D), dtype=np.float32)
        hs[:cfg.NPC] = h[c * cfg.NPC:(c + 1) * cfg.NPC]
        h_t = np.ascontiguousarray(
            hs.reshape(cfg.NW, P, D).transpose(1, 0, 2)
        )
        in1.append({"h_t": h_t, "odr": _per_core_pwd(cfg, odr, c)})

    pk = _pack2(cfg, src_idx, dst_idx, mode)
    iota3 = _iota3_np(cfg, pk["KMAX"])
    in2 = []
    for c in range(cfg.C):
        m = {
            "dstl": np.ascontiguousarray(pk["dstl"][c]),
            "iota3": iota3,
            "idr": _per_core_pwd(cfg, idr, c),
        }
        if mode == "indirect":
            m["offs"] = np.ascontiguousarray(pk["offs"][c])
        else:
            m["idxs"] = np.ascontiguousarray(pk["idxs"][c])
        in2.append(m)
    return pk, in1, in2


def _assemble_h2(cfg, res1):
    h2 = np.zeros((cfg.N, D), dtype=BF)
    for c in range(cfg.C):
        sl = res1.results[c]["h2s"].transpose(1, 0, 2).reshape(cfg.NPC_PAD, D)
        h2[c * cfg.NPC:(c + 1) * cfg.NPC] = sl[:cfg.NPC]
    return h2


def _assemble_out(cfg, res2):
    outs = []
    for c in range(cfg.C):
        o = res2.results[c]["out_t"].reshape(P, cfg.NW, D)
        outs.append(o.transpose(1, 0, 2).reshape(cfg.NPC_PAD, D)[:cfg.NPC])
    return np.concatenate(outs, axis=0)


def run(cfg, inputs, mode=GATHER_MODE, repeat1=1, repeat2=1, parts="all",
        reuse=None):
    pk, in1, in2 = reuse or _prepare(cfg, **inputs, mode=mode)
    nc1 = _build_phase1(cfg, repeat=repeat1)
    res1 = run_bass_kernel_spmd(nc1, in1, core_ids=list(range(cfg.C)))
    h2 = _assemble_h2(cfg, res1)
    for m in in2:
        m["h2"] = h2
    nc2 = _build_phase2(cfg, pk, mode, repeat=repeat2, parts=parts)
    res2 = run_bass_kernel_spmd(nc2, in2, core_ids=list(range(cfg.C)))
    return _assemble_out(cfg, res2), (pk, in1, in2)


def kernel(src_embedding, dst_embedding, src_idx, dst_idx):
    inputs = dict(
        src_embedding=src_embedding, dst_embedding=dst_embedding,
        src_idx=src_idx, dst_idx=dst_idx,
    )
    out, _ = run(CFG, inputs)
    return out
